# revision 6
# baseline (speedup 1.0000x reference)
"""Trainium2 Bass kernel for BucketingBBoxCoder (nms_detection).

Computes, per proposal and per side (l,r,t,d):
  softmax over 7 bucket logits, top-2 values+indices, bucket offset gather,
  bbox decode + clip, and a location confidence score.

Sharding: proposals/cls/offsets split along N across 8 cores (embarrassingly
parallel, no cross-core communication).
"""

import sys

if "/opt/trn_rl_repo" not in sys.path:
    sys.path.insert(0, "/opt/trn_rl_repo")

import numpy as np

import concourse.bass as bass
import concourse.bacc as bacc
import concourse.mybir as mybir
import concourse.tile as tile
from concourse.bass_utils import run_bass_kernel_spmd

# Problem constants (hardcoded per harness contract)
B = 8
N = 131072
SIDE = 7
R = 4 * SIDE  # 28 floats per proposal in cls/offset
NCORES = 8
NS = N // NCORES          # proposals per core per batch
M = B * NS                # total proposals per core (131072)
MAX_W = 1333.0 - 1.0
MAX_H = 800.0 - 1.0
SCALE = 3.0               # SCALE_FACTOR
NB = 14.0                 # NUM_BUCKETS

P = 128                   # partitions
T = 64                    # proposals per partition per tile
TILE_PROPS = P * T        # 8192
NT = M // TILE_PROPS      # 16 tiles per core

F32 = mybir.dt.float32
AX = mybir.AxisListType.X
OP = mybir.AluOpType
AF = mybir.ActivationFunctionType

_BUILT = None


def _build():
    global _BUILT
    if _BUILT is not None:
        return _BUILT

    nc = bacc.Bacc(None, target_bir_lowering=False)

    cls_d = nc.dram_tensor("cls", [M, R], F32, kind="ExternalInput")
    off_d = nc.dram_tensor("off", [M, R], F32, kind="ExternalInput")
    prp_d = nc.dram_tensor("prp", [M, 4], F32, kind="ExternalInput")
    bbx_d = nc.dram_tensor("bbx", [M, 4], F32, kind="ExternalOutput")
    cnf_d = nc.dram_tensor("cnf", [M], F32, kind="ExternalOutput")

    cls_r = cls_d[:, :].rearrange("(i p t) r -> i p (t r)", p=P, t=T)
    off_r = off_d[:, :].rearrange("(i p t) r -> i p (t r)", p=P, t=T)
    prp_r = prp_d[:, :].rearrange("(i p t) r -> i p (t r)", p=P, t=T)
    bbx_r = bbx_d[:, :].rearrange("(i p t) r -> i p (t r)", p=P, t=T)
    cnf_r = cnf_d[:].rearrange("(i p t) -> i p t", p=P, t=T)

    with tile.TileContext(nc) as tc:
        with tc.tile_pool(name="big", bufs=2) as big, \
             tc.tile_pool(name="small", bufs=2) as small:
            for i in range(NT):
                # ---- loads ----
                cls_t = big.tile([P, T * R], F32, tag="cls")
                off_t = big.tile([P, T * R], F32, tag="off")
                prp_t = small.tile([P, T * 4], F32, tag="prp")
                nc.gpsimd.dma_start(out=cls_t[:], in_=cls_r[i])
                nc.gpsimd.dma_start(out=off_t[:], in_=off_r[i])
                nc.gpsimd.dma_start(out=prp_t[:], in_=prp_r[i])

                # ---- softmax pieces ----
                # E = exp(cls)
                e_t = big.tile([P, T * R], F32, tag="e")
                nc.scalar.activation(e_t[:], cls_t[:], AF.Exp)
                e3 = e_t[:].rearrange("p (n s) -> p n s", s=SIDE)  # [P, 4T, 7]

                cls3 = cls_t[:].rearrange("p (n s) -> p n s", s=SIDE)

                z_t = small.tile([P, T * 4], F32, tag="z")
                nc.vector.reduce_sum(z_t[:], e3, axis=AX)
                # top-2 on raw logits (same order as softmax, exact)
                m1_t = small.tile([P, T * 4], F32, tag="m1")
                nc.vector.reduce_max(m1_t[:], cls3, axis=AX)

                # eq1[s] = (cls[s] == M1)
                eq_t = big.tile([P, T * R], F32, tag="eq")
                m1b = m1_t[:].broadcast_to((P, T * 4, SIDE))
                eq3 = eq_t[:].rearrange("p (n s) -> p n s", s=SIDE)
                nc.vector.tensor_tensor(eq3, cls3, m1b, OP.is_equal)

                # i1 = sum_s s*eq1[s]
                i1_t = small.tile([P, T * 4], F32, tag="i1")
                nc.vector.scalar_tensor_tensor(
                    i1_t[:], eq_t[:, 2::SIDE], 2.0, eq_t[:, 1::SIDE], OP.mult, OP.add)
                for s in range(3, SIDE):
                    nc.vector.scalar_tensor_tensor(
                        i1_t[:], eq_t[:, s::SIDE], float(s), i1_t[:], OP.mult, OP.add)

                # Y = cls - 1e30*eq1  (mask out top-1)
                y_t = big.tile([P, T * R], F32, tag="y")
                nc.vector.scalar_tensor_tensor(
                    y_t[:], eq_t[:], -1.0e30, cls_t[:], OP.mult, OP.add)
                y3 = y_t[:].rearrange("p (n s) -> p n s", s=SIDE)

                m2_t = small.tile([P, T * 4], F32, tag="m2")
                nc.vector.reduce_max(m2_t[:], y3, axis=AX)

                # eq2[s] = (Y[s] == M2)
                eq2_t = big.tile([P, T * R], F32, tag="eq2")
                m2b = m2_t[:].broadcast_to((P, T * 4, SIDE))
                eq23 = eq2_t[:].rearrange("p (n s) -> p n s", s=SIDE)
                nc.vector.tensor_tensor(eq23, y3, m2b, OP.is_equal)

                i2_t = small.tile([P, T * 4], F32, tag="i2")
                nc.vector.scalar_tensor_tensor(
                    i2_t[:], eq2_t[:, 2::SIDE], 2.0, eq2_t[:, 1::SIDE], OP.mult, OP.add)
                for s in range(3, SIDE):
                    nc.vector.scalar_tensor_tensor(
                        i2_t[:], eq2_t[:, s::SIDE], float(s), i2_t[:], OP.mult, OP.add)

                # gather offsets at i1: osel = sum_s off[s]*eq1[s]
                om_t = big.tile([P, T * R], F32, tag="om")
                nc.vector.tensor_tensor(om_t[:], eq_t[:], off_t[:], OP.mult)
                om3 = om_t[:].rearrange("p (n s) -> p n s", s=SIDE)
                osel_t = small.tile([P, T * 4], F32, tag="osel")
                nc.vector.reduce_sum(osel_t[:], om3, axis=AX)

                # ---- confidence ----
                # score values: exp(m1)/Z, exp(m2)/Z
                e1_t = small.tile([P, T * 4], F32, tag="e1")
                nc.scalar.activation(e1_t[:], m1_t[:], AF.Exp)
                e2_t = small.tile([P, T * 4], F32, tag="e2")
                nc.scalar.activation(e2_t[:], m2_t[:], AF.Exp)
                zi_t = small.tile([P, T * 4], F32, tag="zi")
                nc.vector.reciprocal(zi_t[:], z_t[:])
                dlt_t = small.tile([P, T * 4], F32, tag="dlt")
                nc.vector.tensor_tensor(dlt_t[:], i1_t[:], i2_t[:], OP.subtract)
                ad_t = small.tile([P, T * 4], F32, tag="ad")
                nc.scalar.activation(ad_t[:], dlt_t[:], AF.Abs)
                # w = (M1 - M2) + M2*|d|  == M1 + M2*(|d|-1)
                u_t = small.tile([P, T * 4], F32, tag="u")
                nc.vector.tensor_tensor(u_t[:], e1_t[:], e2_t[:], OP.subtract)
                v_t = small.tile([P, T * 4], F32, tag="v")
                nc.vector.tensor_tensor(v_t[:], e2_t[:], ad_t[:], OP.mult)
                w_t = small.tile([P, T * 4], F32, tag="w")
                nc.vector.tensor_tensor(w_t[:], u_t[:], v_t[:], OP.add)
                cg_t = small.tile([P, T * 4], F32, tag="cg")
                nc.vector.tensor_tensor(cg_t[:], w_t[:], zi_t[:], OP.mult)
                cg3 = cg_t[:].rearrange("p (t g) -> p t g", g=4)
                cf_t = small.tile([P, T], F32, tag="cf")
                nc.vector.reduce_sum(cf_t[:], cg3, axis=AX)
                cfo_t = small.tile([P, T], F32, tag="cfo")
                nc.scalar.mul(cfo_t[:], cf_t[:], 0.25)
                nc.gpsimd.dma_start(out=cnf_r[i], in_=cfo_t[:])

                # ---- bbox decode ----
                x1 = prp_t[:, 0::4]
                y1 = prp_t[:, 1::4]
                x2 = prp_t[:, 2::4]
                y2 = prp_t[:, 3::4]
                wd_t = small.tile([P, T], F32, tag="wd")
                nc.vector.tensor_tensor(wd_t[:], x2, x1, OP.subtract)
                ht_t = small.tile([P, T], F32, tag="ht")
                nc.vector.tensor_tensor(ht_t[:], y2, y1, OP.subtract)

                # signed bucket sizes per side: (+bw, -bw, +bh, -bh), bw = 3w/14
                bs_t = small.tile([P, T * 4], F32, tag="bs")
                nc.scalar.mul(bs_t[:, 0::4], wd_t[:], SCALE / NB)
                nc.scalar.mul(bs_t[:, 1::4], wd_t[:], -SCALE / NB)
                nc.scalar.mul(bs_t[:, 2::4], ht_t[:], SCALE / NB)
                nc.scalar.mul(bs_t[:, 3::4], ht_t[:], -SCALE / NB)
                # unsigned bucket sizes: (bw, bw, bh, bh)
                bu_t = small.tile([P, T * 4], F32, tag="bu")
                nc.scalar.mul(bu_t[:, 0::4], wd_t[:], SCALE / NB)
                nc.scalar.mul(bu_t[:, 1::4], wd_t[:], SCALE / NB)
                nc.scalar.mul(bu_t[:, 2::4], ht_t[:], SCALE / NB)
                nc.scalar.mul(bu_t[:, 3::4], ht_t[:], SCALE / NB)
                # half-bucket: (+bw/2, -bw/2, +bh/2, -bh/2)
                hs_t = small.tile([P, T * 4], F32, tag="hs")
                nc.scalar.mul(hs_t[:, 0::4], wd_t[:], SCALE / (2 * NB))
                nc.scalar.mul(hs_t[:, 1::4], wd_t[:], -SCALE / (2 * NB))
                nc.scalar.mul(hs_t[:, 2::4], ht_t[:], SCALE / (2 * NB))
                nc.scalar.mul(hs_t[:, 3::4], ht_t[:], -SCALE / (2 * NB))

                # rescaled box edges: px1=2x1-x2, px2=2x2-x1, py1=2y1-y2, py2=2y2-y1
                px_t = small.tile([P, T * 4], F32, tag="px")
                nc.vector.scalar_tensor_tensor(
                    px_t[:, 0::4], x1, 2.0, x2, OP.mult, OP.subtract)
                nc.vector.scalar_tensor_tensor(
                    px_t[:, 1::4], x2, 2.0, x1, OP.mult, OP.subtract)
                nc.vector.scalar_tensor_tensor(
                    px_t[:, 2::4], y1, 2.0, y2, OP.mult, OP.subtract)
                nc.vector.scalar_tensor_tensor(
                    px_t[:, 3::4], y2, 2.0, y1, OP.mult, OP.subtract)

                # out = px + sign*hb + i1*sign*b - osel*b
                pxh_t = small.tile([P, T * 4], F32, tag="pxh")
                nc.vector.tensor_tensor(pxh_t[:], px_t[:], hs_t[:], OP.add)
                mq_t = small.tile([P, T * 4], F32, tag="mq")
                nc.vector.tensor_tensor(mq_t[:], i1_t[:], bs_t[:], OP.mult)
                oq_t = small.tile([P, T * 4], F32, tag="oq")
                nc.vector.tensor_tensor(oq_t[:], osel_t[:], bu_t[:], OP.mult)
                bq_t = small.tile([P, T * 4], F32, tag="bq")
                nc.vector.tensor_tensor(bq_t[:], pxh_t[:], mq_t[:], OP.add)
                bb_t = small.tile([P, T * 4], F32, tag="bb")
                nc.vector.tensor_tensor(bb_t[:], bq_t[:], oq_t[:], OP.subtract)

                # clip + permute (l,r,t,d) -> (x1,y1,x2,y2)
                bbo_t = small.tile([P, T * 4], F32, tag="bbo")
                bb3 = bb_t[:].rearrange("p (t g) -> p t g", g=4)
                bbo3 = bbo_t[:].rearrange("p (t g) -> p t g", g=4)
                nc.vector.tensor_scalar(
                    bbo3[:, :, 0:4:2], bb3[:, :, 0:2], 0.0, MAX_W, OP.max, OP.min)
                nc.vector.tensor_scalar(
                    bbo3[:, :, 1:4:2], bb3[:, :, 2:4], 0.0, MAX_H, OP.max, OP.min)
                nc.gpsimd.dma_start(out=bbx_r[i], in_=bbo_t[:])

    nc.compile()
    _BUILT = nc
    return nc


def kernel(proposals, cls_preds, offset_preds):
    proposals = np.ascontiguousarray(np.asarray(proposals, dtype=np.float32))
    cls_preds = np.ascontiguousarray(np.asarray(cls_preds, dtype=np.float32))
    offset_preds = np.ascontiguousarray(np.asarray(offset_preds, dtype=np.float32))

    cls3 = cls_preds.reshape(B, N, R)
    off3 = offset_preds.reshape(B, N, R)

    in_maps = []
    for k in range(NCORES):
        sl = slice(k * NS, (k + 1) * NS)
        in_maps.append({
            "cls": np.ascontiguousarray(cls3[:, sl].reshape(M, R)),
            "off": np.ascontiguousarray(off3[:, sl].reshape(M, R)),
            "prp": np.ascontiguousarray(proposals[:, sl].reshape(M, 4)),
        })

    nc = _build()
    res = run_bass_kernel_spmd(nc, in_maps, list(range(NCORES)))

    bboxes = np.empty((B, N, 4), dtype=np.float32)
    conf = np.empty((B, N), dtype=np.float32)
    for k in range(NCORES):
        sl = slice(k * NS, (k + 1) * NS)
        bboxes[:, sl] = res.results[k]["bbx"].reshape(B, NS, 4)
        conf[:, sl] = res.results[k]["cnf"].reshape(B, NS)
    return bboxes, conf


# revision 21
# speedup vs baseline: 523.7553x; 523.7553x over previous
"""Trainium2 Bass kernel for BucketingBBoxCoder (nms_detection).

Per proposal and per side (l,r,t,d): softmax over 7 bucket logits, top-2
values+indices, offset gather at top-1 index, bbox decode + clip, and a
location confidence score.

Sharding: N split across 8 cores (embarrassingly parallel).

Engine split (per tile):
  ACT : exp, Sign (second-max mask), Abs, scalar scalings
  DVE : grouped reduces (X-axis), is_equal mask, mask chains (stt)
  Pool: elementwise add/sub/mult passes, clips
"""

import sys

if "/opt/trn_rl_repo" not in sys.path:
    sys.path.insert(0, "/opt/trn_rl_repo")

import numpy as np

import concourse.bass as bass
import concourse.bacc as bacc
import concourse.mybir as mybir
import concourse.tile as tile
from concourse.bass_utils import run_bass_kernel_spmd

B = 8
N = 131072
SIDE = 7
R = 4 * SIDE
NCORES = 8
NS = N // NCORES
M = B * NS                # proposals per core
MAX_W = 1333.0 - 1.0
MAX_H = 800.0 - 1.0
SCALE = 3.0
NB = 14.0

P = 128
T = 64                    # proposals per partition per tile
TILE_PROPS = P * T
NT = M // TILE_PROPS

F32 = mybir.dt.float32
AX = mybir.AxisListType.X
OP = mybir.AluOpType
AF = mybir.ActivationFunctionType

_BUILT = None


def _build():
    global _BUILT
    if _BUILT is not None:
        return _BUILT

    nc = bacc.Bacc(None, target_bir_lowering=False)

    # const AP for the Abs bias (-21): same pattern as Bass.__init__
    _c = nc.alloc_sbuf_tensor("const-float32-neg21", [128, 1], F32)
    nc.gpsimd.memset(_c.ap(), -21.0)
    nc.const_aps.aps[(F32, -21.0)] = _c.ap()
    nc.all_engine_barrier()

    cls_d = nc.dram_tensor("cls", [M, R], F32, kind="ExternalInput")
    off_d = nc.dram_tensor("off", [M, R], F32, kind="ExternalInput")
    prp_d = nc.dram_tensor("prp", [M, 4], F32, kind="ExternalInput")
    bbx_d = nc.dram_tensor("bbx", [M, 4], F32, kind="ExternalOutput")
    cnf_d = nc.dram_tensor("cnf", [M], F32, kind="ExternalOutput")

    cls_r = cls_d[:, :].rearrange("(i p t) r -> i p (t r)", p=P, t=T)
    off_r = off_d[:, :].rearrange("(i p t) r -> i p (t r)", p=P, t=T)
    prp_r = prp_d[:, :].rearrange("(i p t) r -> i p (t r)", p=P, t=T)
    bbx_r = bbx_d[:, :].rearrange("(i p t) r -> i p (t r)", p=P, t=T)
    cnf_r = cnf_d[:].rearrange("(i p t) -> i p t", p=P, t=T)

    with tile.TileContext(nc) as tc:
        with tc.tile_pool(name="bigio", bufs=3) as bigio, \
             tc.tile_pool(name="big", bufs=2) as big, \
             tc.tile_pool(name="small", bufs=2) as small:
            def stage_a(i):
                h = {}
                cls_t = bigio.tile([P, T * R], F32, tag="cls")
                off_t = bigio.tile([P, T * R], F32, tag="off")
                prp_t = small.tile([P, T * 4], F32, tag="prp")
                nc.sync.dma_start(out=cls_t[:], in_=cls_r[i])
                nc.sync.dma_start(out=off_t[:], in_=off_r[i])
                nc.sync.dma_start(out=prp_t[:], in_=prp_r[i])

                cls3 = cls_t[:].rearrange("p (n s) -> p n s", s=SIDE)
                off3 = off_t[:].rearrange("p (n s) -> p n s", s=SIDE)

                # E = exp(cls); Z = sum_s E
                e_t = big.tile([P, T * R], F32, tag="e")
                nc.scalar.activation(e_t[:], cls_t[:], AF.Exp)
                e3 = e_t[:].rearrange("p (n s) -> p n s", s=SIDE)
                h["e3"] = e3

                # top-1 on raw logits (same ordering as softmax)
                m1_t = small.tile([P, T * 4], F32, tag="m1")
                nc.vector.reduce_max(m1_t[:], cls3, axis=AX)
                m1b = m1_t[:].broadcast_to((P, T * 4, SIDE))
                eq_t = big.tile([P, T * R], F32, tag="eq")
                eq3 = eq_t[:].rearrange("p (n s) -> p n s", s=SIDE)
                nc.vector.tensor_tensor(eq3, cls3, m1b, OP.is_equal)

                # gather mask product early so Pool can run ahead
                om_t = big.tile([P, T * R], F32, tag="om")
                om3 = om_t[:].rearrange("p (n s) -> p n s", s=SIDE)
                nc.gpsimd.tensor_tensor(om3, eq3, off3, OP.mult)
                h["om3"] = om3

                # i1 = sum_s s*eq1[s]
                i1_t = small.tile([P, T * 4], F32, tag="i1")
                nc.vector.scalar_tensor_tensor(
                    i1_t[:], eq_t[:, 2::SIDE], 2.0, eq_t[:, 1::SIDE], OP.mult, OP.add)
                for s in range(3, SIDE):
                    nc.vector.scalar_tensor_tensor(
                        i1_t[:], eq_t[:, s::SIDE], float(s), i1_t[:], OP.mult, OP.add)

                # Y = cls - 1e30*eq1; M2 = max_s Y  (second max)
                y_t = big.tile([P, T * R], F32, tag="y")
                nc.vector.scalar_tensor_tensor(
                    y_t[:], eq_t[:], -1.0e30, cls_t[:], OP.mult, OP.add)
                y3 = y_t[:].rearrange("p (n s) -> p n s", s=SIDE)
                m2_t = small.tile([P, T * 4], F32, tag="m2")
                nc.vector.reduce_max(m2_t[:], y3, axis=AX)

                # second-index mask via sign: sgn2 = sign(Y - M2) in {-1, 0}
                m2b = m2_t[:].broadcast_to((P, T * 4, SIDE))
                df2_t = big.tile([P, T * R], F32, tag="df2")
                df23 = df2_t[:].rearrange("p (n s) -> p n s", s=SIDE)
                nc.gpsimd.tensor_tensor(df23, y3, m2b, OP.subtract)
                sg2_t = big.tile([P, T * R], F32, tag="sg2")
                nc.scalar.activation(sg2_t[:], df2_t[:], AF.Sign)
                h["sg2_t"] = sg2_t

                # score values + 1/Z
                e1_t = small.tile([P, T * 4], F32, tag="e1")
                nc.scalar.activation(e1_t[:], m1_t[:], AF.Exp)
                e2_t = small.tile([P, T * 4], F32, tag="e2")
                nc.scalar.activation(e2_t[:], m2_t[:], AF.Exp)
                h.update(i1_t=i1_t, e1_t=e1_t, e2_t=e2_t)

                # ---- bbox geometry (independent of cls chain) ----
                x1 = prp_t[:, 0::4]
                y1 = prp_t[:, 1::4]
                x2 = prp_t[:, 2::4]
                y2 = prp_t[:, 3::4]
                wd_t = small.tile([P, T], F32, tag="wd")
                nc.gpsimd.tensor_tensor(wd_t[:], x2, x1, OP.subtract)
                ht_t = small.tile([P, T], F32, tag="ht")
                nc.gpsimd.tensor_tensor(ht_t[:], y2, y1, OP.subtract)

                # NOTE: geometry tiles below use slot order (l,t,r,d)
                # (= proposal coord order x1,y1,x2,y2) instead of (l,r,t,d)
                bs_t = small.tile([P, T * 4], F32, tag="bs")
                nc.scalar.mul(bs_t[:, 0::4], wd_t[:], SCALE / NB)
                nc.scalar.mul(bs_t[:, 1::4], ht_t[:], SCALE / NB)
                nc.scalar.mul(bs_t[:, 2::4], wd_t[:], -SCALE / NB)
                nc.scalar.mul(bs_t[:, 3::4], ht_t[:], -SCALE / NB)
                bu_t = small.tile([P, T * 4], F32, tag="bu")
                nc.scalar.mul(bu_t[:, 0::4], wd_t[:], SCALE / NB)
                nc.scalar.mul(bu_t[:, 1::4], ht_t[:], SCALE / NB)
                nc.scalar.mul(bu_t[:, 2::4], wd_t[:], SCALE / NB)
                nc.scalar.mul(bu_t[:, 3::4], ht_t[:], SCALE / NB)
                hs_t = small.tile([P, T * 4], F32, tag="hs")
                nc.scalar.mul(hs_t[:, 0::4], wd_t[:], SCALE / (2 * NB))
                nc.scalar.mul(hs_t[:, 1::4], ht_t[:], SCALE / (2 * NB))
                nc.scalar.mul(hs_t[:, 2::4], wd_t[:], -SCALE / (2 * NB))
                nc.scalar.mul(hs_t[:, 3::4], ht_t[:], -SCALE / (2 * NB))

                # px_j = 2*prp_j - prp_{j xor 2} for all 4 coords in ONE op:
                # partner view swaps the (x1,y1)/(x2,y2) halves via a
                # reversed middle dim
                prp3 = prp_t[:].rearrange("p (t g) -> p t g", g=4)
                px_t = small.tile([P, T * 4], F32, tag="px")
                px3 = px_t[:].rearrange("p (t g) -> p t g", g=4)
                nc.vector.scalar_tensor_tensor(
                    px3[:, :, 0:2], prp3[:, :, 0:2], 2.0, prp3[:, :, 2:4],
                    OP.mult, OP.subtract)
                nc.vector.scalar_tensor_tensor(
                    px3[:, :, 2:4], prp3[:, :, 2:4], 2.0, prp3[:, :, 0:2],
                    OP.mult, OP.subtract)
                pxh_t = small.tile([P, T * 4], F32, tag="pxh")
                nc.gpsimd.tensor_tensor(pxh_t[:], px_t[:], hs_t[:], OP.add)
                h.update(bs_t=bs_t, bu_t=bu_t, pxh_t=pxh_t)
                return h

            def stage_b(i, h):
                # Z = sum_s E (exp ran in stage A)
                z_t = small.tile([P, T * 4], F32, tag="z")
                nc.vector.reduce_sum(z_t[:], h["e3"], axis=AX)
                zi_t = small.tile([P, T * 4], F32, tag="zi")
                nc.vector.reciprocal(zi_t[:], z_t[:])

                # osel = sum_s off[s]*eq1[s]  (om produced in stage A)
                osel_t = small.tile([P, T * 4], F32, tag="osel")
                nc.vector.reduce_sum(osel_t[:], h["om3"], axis=AX)

                # i2'' = sum_s s*sgn2[s]   (true i2 = 21 + i2'')
                sg2_t = h["sg2_t"]
                i2_t = small.tile([P, T * 4], F32, tag="i2")
                nc.vector.scalar_tensor_tensor(
                    i2_t[:], sg2_t[:, 2::SIDE], 2.0, sg2_t[:, 1::SIDE], OP.mult, OP.add)
                for s in range(3, SIDE):
                    nc.vector.scalar_tensor_tensor(
                        i2_t[:], sg2_t[:, s::SIDE], float(s), i2_t[:], OP.mult, OP.add)

                i1_t = h["i1_t"]
                # dlt_true = i1 - (21 + i2''); ad = |i1 - i2'' - 21| via bias
                dr_t = small.tile([P, T * 4], F32, tag="dr")
                nc.gpsimd.tensor_tensor(dr_t[:], i1_t[:], i2_t[:], OP.subtract)
                ad_t = small.tile([P, T * 4], F32, tag="ad")
                nc.scalar.activation(ad_t[:], dr_t[:], AF.Abs, bias=-21.0)
                # conf_g = (e1 - e2 + e2*|dlt|) / Z
                u_t = small.tile([P, T * 4], F32, tag="u")
                nc.gpsimd.tensor_tensor(u_t[:], h["e1_t"][:], h["e2_t"][:], OP.subtract)
                v_t = small.tile([P, T * 4], F32, tag="v")
                nc.gpsimd.tensor_tensor(v_t[:], h["e2_t"][:], ad_t[:], OP.mult)
                w_t = small.tile([P, T * 4], F32, tag="w")
                nc.gpsimd.tensor_tensor(w_t[:], u_t[:], v_t[:], OP.add)
                cg_t = small.tile([P, T * 4], F32, tag="cg")
                nc.gpsimd.tensor_tensor(cg_t[:], w_t[:], zi_t[:], OP.mult)
                cg3 = cg_t[:].rearrange("p (t g) -> p t g", g=4)
                cf_t = small.tile([P, T], F32, tag="cf")
                nc.vector.reduce_sum(cf_t[:], cg3, axis=AX)
                cfo_t = small.tile([P, T], F32, tag="cfo")
                nc.scalar.mul(cfo_t[:], cf_t[:], 0.25)
                nc.sync.dma_start(out=cnf_r[i], in_=cfo_t[:])

                # out = pxh + i1*bs - osel*bu  (geometry is in (l,t,r,d)
                # slot order; read i1/osel through a permuted view: group
                # sequence (0,2,1,3) via dims [2(stride 1), 2(stride 2)])
                def ltrd(t):
                    return t[:].rearrange("p (t v u) -> p t u v", u=2, v=2)
                mq_t = small.tile([P, T * 4], F32, tag="mq")
                mq4 = mq_t[:].rearrange("p (t g) -> p t g", g=4)
                bs4 = h["bs_t"][:].rearrange("p (t g) -> p t g", g=4)
                nc.gpsimd.tensor_tensor(
                    mq4.rearrange("p t (u v) -> p t u v", u=2),
                    ltrd(i1_t),
                    bs4.rearrange("p t (u v) -> p t u v", u=2), OP.mult)
                oq_t = small.tile([P, T * 4], F32, tag="oq")
                oq4 = oq_t[:].rearrange("p (t g) -> p t g", g=4)
                bu4 = h["bu_t"][:].rearrange("p (t g) -> p t g", g=4)
                nc.gpsimd.tensor_tensor(
                    oq4.rearrange("p t (u v) -> p t u v", u=2),
                    ltrd(osel_t),
                    bu4.rearrange("p t (u v) -> p t u v", u=2), OP.mult)
                bq_t = small.tile([P, T * 4], F32, tag="bq")
                nc.gpsimd.tensor_tensor(bq_t[:], h["pxh_t"][:], mq_t[:], OP.add)
                bb_t = small.tile([P, T * 4], F32, tag="bb")
                nc.gpsimd.tensor_tensor(bb_t[:], bq_t[:], oq_t[:], OP.subtract)

                # bb slots are (l,t,r,d) == output coord order (x1,y1,x2,y2)
                bbo_t = small.tile([P, T * 4], F32, tag="bbo")
                bb3 = bb_t[:].rearrange("p (t g) -> p t g", g=4)
                bbo3 = bbo_t[:].rearrange("p (t g) -> p t g", g=4)
                nc.gpsimd.tensor_scalar(
                    bbo3[:, :, 0:4:2], bb3[:, :, 0:4:2], 0.0, MAX_W, OP.max, OP.min)
                nc.gpsimd.tensor_scalar(
                    bbo3[:, :, 1:4:2], bb3[:, :, 1:4:2], 0.0, MAX_H, OP.max, OP.min)
                nc.sync.dma_start(out=bbx_r[i], in_=bbo_t[:])

            prev = None
            for i in range(NT):
                h = stage_a(i)
                if prev is not None:
                    stage_b(i - 1, prev)
                prev = h
            stage_b(NT - 1, prev)

    nc.compile()
    _BUILT = nc
    return nc


def kernel(proposals, cls_preds, offset_preds):
    proposals = np.ascontiguousarray(np.asarray(proposals, dtype=np.float32))
    cls_preds = np.ascontiguousarray(np.asarray(cls_preds, dtype=np.float32))
    offset_preds = np.ascontiguousarray(np.asarray(offset_preds, dtype=np.float32))

    cls3 = cls_preds.reshape(B, N, R)
    off3 = offset_preds.reshape(B, N, R)

    in_maps = []
    for k in range(NCORES):
        sl = slice(k * NS, (k + 1) * NS)
        in_maps.append({
            "cls": np.ascontiguousarray(cls3[:, sl].reshape(M, R)),
            "off": np.ascontiguousarray(off3[:, sl].reshape(M, R)),
            "prp": np.ascontiguousarray(proposals[:, sl].reshape(M, 4)),
        })

    nc = _build()
    res = run_bass_kernel_spmd(nc, in_maps, list(range(NCORES)))

    bboxes = np.empty((B, N, 4), dtype=np.float32)
    conf = np.empty((B, N), dtype=np.float32)
    for k in range(NCORES):
        sl = slice(k * NS, (k + 1) * NS)
        bboxes[:, sl] = res.results[k]["bbx"].reshape(B, NS, 4)
        conf[:, sl] = res.results[k]["cnf"].reshape(B, NS)
    return bboxes, conf


# revision 22
# speedup vs baseline: 563.0498x; 1.0750x over previous
"""Trainium2 Bass kernel for BucketingBBoxCoder (nms_detection).

Per proposal and per side (l,r,t,d): softmax over 7 bucket logits, top-2
values+indices, offset gather at top-1 index, bbox decode + clip, and a
location confidence score.

Sharding: N split across 8 cores (embarrassingly parallel).

Engine split (per tile):
  ACT : exp, Sign (second-max mask), Abs, scalar scalings
  DVE : grouped reduces (X-axis), is_equal mask, mask chains (stt)
  Pool: elementwise add/sub/mult passes, clips
"""

import sys

if "/opt/trn_rl_repo" not in sys.path:
    sys.path.insert(0, "/opt/trn_rl_repo")

import numpy as np

import concourse.bass as bass
import concourse.bacc as bacc
import concourse.mybir as mybir
import concourse.tile as tile
from concourse.bass_utils import run_bass_kernel_spmd

B = 8
N = 131072
SIDE = 7
R = 4 * SIDE
NCORES = 8
NS = N // NCORES
M = B * NS                # proposals per core
MAX_W = 1333.0 - 1.0
MAX_H = 800.0 - 1.0
SCALE = 3.0
NB = 14.0

P = 128
T = 64                    # proposals per partition per tile
TILE_PROPS = P * T
NT = M // TILE_PROPS

F32 = mybir.dt.float32
AX = mybir.AxisListType.X
OP = mybir.AluOpType
AF = mybir.ActivationFunctionType

_BUILT = None


def _build():
    global _BUILT
    if _BUILT is not None:
        return _BUILT

    nc = bacc.Bacc(None, target_bir_lowering=False)

    # const AP for the Abs bias (-21): same pattern as Bass.__init__
    _c = nc.alloc_sbuf_tensor("const-float32-neg21", [128, 1], F32)
    nc.gpsimd.memset(_c.ap(), -21.0)
    nc.const_aps.aps[(F32, -21.0)] = _c.ap()
    nc.all_engine_barrier()

    cls_d = nc.dram_tensor("cls", [M, R], F32, kind="ExternalInput")
    off_d = nc.dram_tensor("off", [M, R], F32, kind="ExternalInput")
    prp_d = nc.dram_tensor("prp", [M, 4], F32, kind="ExternalInput")
    bbx_d = nc.dram_tensor("bbx", [M, 4], F32, kind="ExternalOutput")
    cnf_d = nc.dram_tensor("cnf", [M], F32, kind="ExternalOutput")

    cls_r = cls_d[:, :].rearrange("(i p t) r -> i p (t r)", p=P, t=T)
    off_r = off_d[:, :].rearrange("(i p t) r -> i p (t r)", p=P, t=T)
    prp_r = prp_d[:, :].rearrange("(i p t) r -> i p (t r)", p=P, t=T)
    bbx_r = bbx_d[:, :].rearrange("(i p t) r -> i p (t r)", p=P, t=T)
    cnf_r = cnf_d[:].rearrange("(i p t) -> i p t", p=P, t=T)

    with tile.TileContext(nc) as tc:
        with tc.tile_pool(name="bigio", bufs=3) as bigio, \
             tc.tile_pool(name="big", bufs=2) as big, \
             tc.tile_pool(name="small", bufs=2) as small:
            def stage_a(i):
                h = {}
                cls_t = bigio.tile([P, T * R], F32, tag="cls")
                off_t = bigio.tile([P, T * R], F32, tag="off")
                prp_t = small.tile([P, T * 4], F32, tag="prp")
                nc.sync.dma_start(out=cls_t[:], in_=cls_r[i])
                nc.sync.dma_start(out=off_t[:], in_=off_r[i])
                nc.sync.dma_start(out=prp_t[:], in_=prp_r[i])

                cls3 = cls_t[:].rearrange("p (n s) -> p n s", s=SIDE)
                off3 = off_t[:].rearrange("p (n s) -> p n s", s=SIDE)

                # E = exp(cls); Z = sum_s E
                e_t = big.tile([P, T * R], F32, tag="e")
                nc.scalar.activation(e_t[:], cls_t[:], AF.Exp)
                e3 = e_t[:].rearrange("p (n s) -> p n s", s=SIDE)
                h["e3"] = e3

                # top-1 on raw logits (same ordering as softmax)
                m1_t = small.tile([P, T * 4], F32, tag="m1")
                nc.vector.reduce_max(m1_t[:], cls3, axis=AX)
                m1b = m1_t[:].broadcast_to((P, T * 4, SIDE))
                mk_t = big.tile([P, 2 * T * R], F32, tag="mk")
                eq_h = mk_t[:, 0:T * R]
                eq3 = eq_h.rearrange("p (n s) -> p n s", s=SIDE)
                nc.vector.tensor_tensor(eq3, cls3, m1b, OP.is_equal)

                # gather mask product early so Pool can run ahead
                om_t = big.tile([P, T * R], F32, tag="om")
                om3 = om_t[:].rearrange("p (n s) -> p n s", s=SIDE)
                nc.gpsimd.tensor_tensor(om3, eq3, off3, OP.mult)
                h["om3"] = om3

                # Y = cls - 1e30*eq1; M2 = max_s Y  (second max)
                y_t = big.tile([P, T * R], F32, tag="y")
                nc.vector.scalar_tensor_tensor(
                    y_t[:], eq_h, -1.0e30, cls_t[:], OP.mult, OP.add)
                y3 = y_t[:].rearrange("p (n s) -> p n s", s=SIDE)
                m2_t = small.tile([P, T * 4], F32, tag="m2")
                nc.vector.reduce_max(m2_t[:], y3, axis=AX)

                # second-index mask via sign: sgn2 = sign(Y - M2) in {-1, 0}
                m2b = m2_t[:].broadcast_to((P, T * 4, SIDE))
                df2_t = big.tile([P, T * R], F32, tag="df2")
                df23 = df2_t[:].rearrange("p (n s) -> p n s", s=SIDE)
                nc.gpsimd.tensor_tensor(df23, y3, m2b, OP.subtract)
                nc.scalar.activation(mk_t[:, T * R:], df2_t[:], AF.Sign)
                h["mk_t"] = mk_t

                # score values + 1/Z
                e1_t = small.tile([P, T * 4], F32, tag="e1")
                nc.scalar.activation(e1_t[:], m1_t[:], AF.Exp)
                e2_t = small.tile([P, T * 4], F32, tag="e2")
                nc.scalar.activation(e2_t[:], m2_t[:], AF.Exp)
                h.update(e1_t=e1_t, e2_t=e2_t)

                # ---- bbox geometry (independent of cls chain) ----
                x1 = prp_t[:, 0::4]
                y1 = prp_t[:, 1::4]
                x2 = prp_t[:, 2::4]
                y2 = prp_t[:, 3::4]
                wd_t = small.tile([P, T], F32, tag="wd")
                nc.gpsimd.tensor_tensor(wd_t[:], x2, x1, OP.subtract)
                ht_t = small.tile([P, T], F32, tag="ht")
                nc.gpsimd.tensor_tensor(ht_t[:], y2, y1, OP.subtract)

                # NOTE: geometry tiles below use slot order (l,t,r,d)
                # (= proposal coord order x1,y1,x2,y2) instead of (l,r,t,d)
                bs_t = small.tile([P, T * 4], F32, tag="bs")
                nc.scalar.mul(bs_t[:, 0::4], wd_t[:], SCALE / NB)
                nc.scalar.mul(bs_t[:, 1::4], ht_t[:], SCALE / NB)
                nc.scalar.mul(bs_t[:, 2::4], wd_t[:], -SCALE / NB)
                nc.scalar.mul(bs_t[:, 3::4], ht_t[:], -SCALE / NB)
                bu_t = small.tile([P, T * 4], F32, tag="bu")
                nc.scalar.mul(bu_t[:, 0::4], wd_t[:], SCALE / NB)
                nc.scalar.mul(bu_t[:, 1::4], ht_t[:], SCALE / NB)
                nc.scalar.mul(bu_t[:, 2::4], wd_t[:], SCALE / NB)
                nc.scalar.mul(bu_t[:, 3::4], ht_t[:], SCALE / NB)
                hs_t = small.tile([P, T * 4], F32, tag="hs")
                nc.scalar.mul(hs_t[:, 0::4], wd_t[:], SCALE / (2 * NB))
                nc.scalar.mul(hs_t[:, 1::4], ht_t[:], SCALE / (2 * NB))
                nc.scalar.mul(hs_t[:, 2::4], wd_t[:], -SCALE / (2 * NB))
                nc.scalar.mul(hs_t[:, 3::4], ht_t[:], -SCALE / (2 * NB))

                # px_j = 2*prp_j - prp_{j xor 2} for all 4 coords in ONE op:
                # partner view swaps the (x1,y1)/(x2,y2) halves via a
                # reversed middle dim
                prp3 = prp_t[:].rearrange("p (t g) -> p t g", g=4)
                px_t = small.tile([P, T * 4], F32, tag="px")
                px3 = px_t[:].rearrange("p (t g) -> p t g", g=4)
                nc.vector.scalar_tensor_tensor(
                    px3[:, :, 0:2], prp3[:, :, 0:2], 2.0, prp3[:, :, 2:4],
                    OP.mult, OP.subtract)
                nc.vector.scalar_tensor_tensor(
                    px3[:, :, 2:4], prp3[:, :, 2:4], 2.0, prp3[:, :, 0:2],
                    OP.mult, OP.subtract)
                pxh_t = small.tile([P, T * 4], F32, tag="pxh")
                nc.gpsimd.tensor_tensor(pxh_t[:], px_t[:], hs_t[:], OP.add)
                h.update(bs_t=bs_t, bu_t=bu_t, pxh_t=pxh_t)
                return h

            def stage_b(i, h):
                # Z = sum_s E (exp ran in stage A)
                z_t = small.tile([P, T * 4], F32, tag="z")
                nc.vector.reduce_sum(z_t[:], h["e3"], axis=AX)
                zi_t = small.tile([P, T * 4], F32, tag="zi")
                nc.vector.reciprocal(zi_t[:], z_t[:])

                # osel = sum_s off[s]*eq1[s]  (om produced in stage A)
                osel_t = small.tile([P, T * 4], F32, tag="osel")
                nc.vector.reduce_sum(osel_t[:], h["om3"], axis=AX)

                # fused index chains: halves of mk are (eq1 | sgn2);
                # one stt per s computes [i1 | i2''] together
                mk3 = h["mk_t"][:].rearrange("p (hh q) -> p hh q", hh=2)
                i12_t = small.tile([P, 2 * T * 4], F32, tag="i12")
                i12_3 = i12_t[:].rearrange("p (hh q) -> p hh q", hh=2)
                nc.vector.scalar_tensor_tensor(
                    i12_3, mk3[:, :, 2::SIDE], 2.0, mk3[:, :, 1::SIDE],
                    OP.mult, OP.add)
                for s in range(3, SIDE):
                    nc.vector.scalar_tensor_tensor(
                        i12_3, mk3[:, :, s::SIDE], float(s), i12_3,
                        OP.mult, OP.add)
                i1_t = i12_t[:, 0:T * 4]
                i2_t = i12_t[:, T * 4:]
                # dlt_true = i1 - (21 + i2''); ad = |i1 - i2'' - 21| via bias
                dr_t = small.tile([P, T * 4], F32, tag="dr")
                nc.gpsimd.tensor_tensor(dr_t[:], i1_t, i2_t, OP.subtract)
                ad_t = small.tile([P, T * 4], F32, tag="ad")
                nc.scalar.activation(ad_t[:], dr_t[:], AF.Abs, bias=-21.0)
                # conf_g = (e1 - e2 + e2*|dlt|) / Z
                u_t = small.tile([P, T * 4], F32, tag="u")
                nc.gpsimd.tensor_tensor(u_t[:], h["e1_t"][:], h["e2_t"][:], OP.subtract)
                v_t = small.tile([P, T * 4], F32, tag="v")
                nc.gpsimd.tensor_tensor(v_t[:], h["e2_t"][:], ad_t[:], OP.mult)
                w_t = small.tile([P, T * 4], F32, tag="w")
                nc.gpsimd.tensor_tensor(w_t[:], u_t[:], v_t[:], OP.add)
                cg_t = small.tile([P, T * 4], F32, tag="cg")
                nc.gpsimd.tensor_tensor(cg_t[:], w_t[:], zi_t[:], OP.mult)
                cg3 = cg_t[:].rearrange("p (t g) -> p t g", g=4)
                cf_t = small.tile([P, T], F32, tag="cf")
                nc.vector.reduce_sum(cf_t[:], cg3, axis=AX)
                cfo_t = small.tile([P, T], F32, tag="cfo")
                nc.scalar.mul(cfo_t[:], cf_t[:], 0.25)
                nc.sync.dma_start(out=cnf_r[i], in_=cfo_t[:])

                # out = pxh + i1*bs - osel*bu  (geometry is in (l,t,r,d)
                # slot order; read i1/osel through a permuted view: group
                # sequence (0,2,1,3) via dims [2(stride 1), 2(stride 2)])
                mq_t = small.tile([P, T * 4], F32, tag="mq")
                mq4 = mq_t[:].rearrange("p (t g) -> p t g", g=4)
                bs4 = h["bs_t"][:].rearrange("p (t g) -> p t g", g=4)
                nc.gpsimd.tensor_tensor(
                    mq4.rearrange("p t (u v) -> p t u v", u=2),
                    i1_t.rearrange("p (t v u) -> p t u v", u=2, v=2),
                    bs4.rearrange("p t (u v) -> p t u v", u=2), OP.mult)
                oq_t = small.tile([P, T * 4], F32, tag="oq")
                oq4 = oq_t[:].rearrange("p (t g) -> p t g", g=4)
                bu4 = h["bu_t"][:].rearrange("p (t g) -> p t g", g=4)
                nc.gpsimd.tensor_tensor(
                    oq4.rearrange("p t (u v) -> p t u v", u=2),
                    osel_t[:].rearrange("p (t v u) -> p t u v", u=2, v=2),
                    bu4.rearrange("p t (u v) -> p t u v", u=2), OP.mult)
                bq_t = small.tile([P, T * 4], F32, tag="bq")
                nc.gpsimd.tensor_tensor(bq_t[:], h["pxh_t"][:], mq_t[:], OP.add)
                bb_t = small.tile([P, T * 4], F32, tag="bb")
                nc.gpsimd.tensor_tensor(bb_t[:], bq_t[:], oq_t[:], OP.subtract)

                # bb slots are (l,t,r,d) == output coord order (x1,y1,x2,y2)
                bbo_t = small.tile([P, T * 4], F32, tag="bbo")
                bb3 = bb_t[:].rearrange("p (t g) -> p t g", g=4)
                bbo3 = bbo_t[:].rearrange("p (t g) -> p t g", g=4)
                nc.gpsimd.tensor_scalar(
                    bbo3[:, :, 0:4:2], bb3[:, :, 0:4:2], 0.0, MAX_W, OP.max, OP.min)
                nc.gpsimd.tensor_scalar(
                    bbo3[:, :, 1:4:2], bb3[:, :, 1:4:2], 0.0, MAX_H, OP.max, OP.min)
                nc.sync.dma_start(out=bbx_r[i], in_=bbo_t[:])

            prev = None
            for i in range(NT):
                h = stage_a(i)
                if prev is not None:
                    stage_b(i - 1, prev)
                prev = h
            stage_b(NT - 1, prev)

    nc.compile()
    _BUILT = nc
    return nc


def kernel(proposals, cls_preds, offset_preds):
    proposals = np.ascontiguousarray(np.asarray(proposals, dtype=np.float32))
    cls_preds = np.ascontiguousarray(np.asarray(cls_preds, dtype=np.float32))
    offset_preds = np.ascontiguousarray(np.asarray(offset_preds, dtype=np.float32))

    cls3 = cls_preds.reshape(B, N, R)
    off3 = offset_preds.reshape(B, N, R)

    in_maps = []
    for k in range(NCORES):
        sl = slice(k * NS, (k + 1) * NS)
        in_maps.append({
            "cls": np.ascontiguousarray(cls3[:, sl].reshape(M, R)),
            "off": np.ascontiguousarray(off3[:, sl].reshape(M, R)),
            "prp": np.ascontiguousarray(proposals[:, sl].reshape(M, 4)),
        })

    nc = _build()
    res = run_bass_kernel_spmd(nc, in_maps, list(range(NCORES)))

    bboxes = np.empty((B, N, 4), dtype=np.float32)
    conf = np.empty((B, N), dtype=np.float32)
    for k in range(NCORES):
        sl = slice(k * NS, (k + 1) * NS)
        bboxes[:, sl] = res.results[k]["bbx"].reshape(B, NS, 4)
        conf[:, sl] = res.results[k]["cnf"].reshape(B, NS)
    return bboxes, conf
